# revision 50
# baseline (speedup 1.0000x reference)
"""Tensor-parallel multi-head attention (RoPE) kernel for 8 Trainium2 cores.

Shapes (hardcoded): x [2, 2048, 1024], 16 heads x head_dim 64.
Sharding: core c -> batch b = c//4, head-group hg = c%4 (4 heads = 256
projection columns). Each core computes q/k/v projections for its head
columns, RoPE, attention, and a partial out-projection over its 256 rows
of o_w; the host sums the 4 partials per batch and adds o_b (plus the
v_b @ o_w term, which passes through attention linearly).

Device-side design:
  - ALL matmuls in bf16 (fp32 PSUM accumulation). fp32r matmuls sit in
    the power-throttled activity class (PE duty-cycled to ~50%); bf16
    mostly escapes the throttle and enables fast weight load.
  - qT/kT stored transposed [head_dim on partitions, tokens on free]
    so QK^T contracts over partitions directly. scores computed
    transposed S^T[k, q]; softmax max-subtraction skipped (scores are
    O(+-9), fp32 exp handles that exactly).
  - PV uses stationary [V | 1] so one accumulation produces both the
    unnormalized output and the softmax denominators (row 64). Head
    groups in Vsb are padded to stride 66 so the V-copy and the
    ones-column DMA never share a 32-bit SBUF word (bf16 sub-word
    tearing between engines is a real-HW race CoreSim cannot see).
  - The scalar engine's exp stream (4 heads x 2048^2 scores at ~1.2
    GHz, ~150us) is the kernel's floor, so everything is organized to
    start it early and never let it starve: attention runs as 8
    "bursts" (head x q-half) of 16 k-tiles; the scores go to a 3-deep
    [128, 1024] PSUM pool so the S matmuls prefetch ahead of exp; each
    burst's PV matmuls chase the PREVIOUS burst's exps; and the
    leftover projections (V tiles, k e1) are drip-fed one tile per
    k-tile step as PE filler inside the first bursts instead of
    delaying the exp stream by a whole projection phase.
  - softmax normalize: DVE copies raw output+denoms out of PSUM
    immediately (frees the accumulator), the denominator reciprocal is
    DMA-scattered across 128 partitions (a [1, N] reciprocal is ~8
    cyc/elem on a single DVE lane), inverted, gathered back, gpsimd-
    broadcast, and multiplied -- all overlapped with the next burst.
"""

import sys
import numpy as np

for p in ("/opt/trn_rl_repo", "/root/.axon_site/_ro/trn_rl_repo"):
    if p not in sys.path:
        sys.path.insert(0, p)

B, L, D = 2, 2048, 1024
H, HD = 16, 64
NCORES = 8
HG = 4                  # head-groups == cores per batch
EL = D // HG            # 256 projection columns per core
ET = EL // 128          # 2 e-tiles
DT = D // 128           # 8 d-tiles
TT = L // 128           # 16 token tiles
NH = H // HG            # 4 heads per core
QH = 1024               # attention q processed in halves

_cache = {}

def _build():
    import concourse.mybir as mybir
    from concourse import bacc, tile

    F32 = mybir.dt.float32
    BF16 = mybir.dt.bfloat16
    AF = mybir.ActivationFunctionType

    nc = bacc.Bacc("TRN2", target_bir_lowering=False, debug=False,
                   num_devices=NCORES)

    xT = nc.dram_tensor("xT", [D, L], BF16, kind="ExternalInput").ap()
    wq = nc.dram_tensor("wq", [D, EL], BF16, kind="ExternalInput").ap()
    wk = nc.dram_tensor("wk", [D, EL], BF16, kind="ExternalInput").ap()
    wv = nc.dram_tensor("wv", [D, EL], BF16, kind="ExternalInput").ap()
    wo = nc.dram_tensor("wo", [EL, D], BF16, kind="ExternalInput").ap()
    bq = nc.dram_tensor("bq", [ET, 128, 1], F32, kind="ExternalInput").ap()
    bk = nc.dram_tensor("bk", [ET, 128, 1], F32, kind="ExternalInput").ap()
    cosb = nc.dram_tensor("cosb", [128, L], BF16, kind="ExternalInput").ap()
    onesd = nc.dram_tensor("onesd", [128, NH, 1], BF16, kind="ExternalInput").ap()
    sinb = nc.dram_tensor("sinb", [128, L], BF16, kind="ExternalInput").ap()
    outT = nc.dram_tensor("outT", [D, L], BF16, kind="ExternalOutput").ap()

    with tile.TileContext(nc) as tc:
        with (
            tc.tile_pool(name="persist", bufs=1) as P,
            tc.tile_pool(name="xw", bufs=1) as XW,
            tc.tile_pool(name="ropet", bufs=2) as RT,
            tc.tile_pool(name="ps", bufs=3, space="PSUM") as PS,
            tc.tile_pool(name="po", bufs=1, space="PSUM") as PO,
            tc.tile_pool(name="esb", bufs=20) as EP,
            tc.tile_pool(name="smallsb", bufs=2) as SS,
            tc.tile_pool(name="od", bufs=2) as OD,
        ):
            qT = [P.tile([128, L], BF16, name=f"qT{e}") for e in range(ET)]
            kT = [P.tile([128, L], BF16, name=f"kT{e}") for e in range(ET)]
            # head group stride 66 (not 65): 4B-aligned groups
            Vsb = [P.tile([128, NH * 66], BF16, name=f"V{t}") for t in range(TT)]
            ao = [P.tile([128, L], BF16, name=f"ao{e}") for e in range(ET)]

            xts = [XW.tile([128, L], BF16, name=f"x{d}") for d in range(DT)]
            wqs = [XW.tile([128, EL], BF16, name=f"wq{d}") for d in range(DT)]
            wks = [XW.tile([128, EL], BF16, name=f"wk{d}") for d in range(DT)]
            wvs = [XW.tile([128, EL], BF16, name=f"wv{d}") for d in range(DT)]
            wos = [XW.tile([128, D], BF16, name=f"wo{e}") for e in range(ET)]
            # Input DMA issue is spread over all three DGE sequencers —
            # each dma_start costs ~1.2us of ISSUE time on its sequencer,
            # so a single queue serializes the whole input load (~45us).
            # sync carries x (the bandwidth gate, in halves so transfers
            # parallelize across DMA engines), ACT carries the q weights
            # (it is idle until the first exp), gpsimd carries the rest.
            # x: all first-halves before second-halves, so the first
            # projection halves (and their RoPE) complete early.
            for d in range(DT):
                nc.sync.dma_start(xts[d][:, 0:1024],
                                  xT[d * 128:(d + 1) * 128, 0:1024])
            for d in range(DT):
                nc.sync.dma_start(xts[d][:, 1024:L],
                                  xT[d * 128:(d + 1) * 128, 1024:L])
            for d in range(DT):
                nc.scalar.dma_start(wqs[d][:], wq[d * 128:(d + 1) * 128, :])
            cosbt = XW.tile([128, L], BF16)
            sinbt = XW.tile([128, L], BF16)
            # quartered: a single 512KB DMA runs ~23us on one DMA engine,
            # and RoPE (on the exp-stream critical path) needs these early
            for c4 in range(0, L, 512):
                nc.gpsimd.dma_start(cosbt[:, c4:c4 + 512],
                                    cosb[:, c4:c4 + 512])
                nc.gpsimd.dma_start(sinbt[:, c4:c4 + 512],
                                    sinb[:, c4:c4 + 512])
            for d in range(DT):
                nc.gpsimd.dma_start(wks[d][:], wk[d * 128:(d + 1) * 128, :])
            for d in range(DT):
                nc.gpsimd.dma_start(wvs[d][:], wv[d * 128:(d + 1) * 128, :])
            for e in range(ET):
                nc.gpsimd.dma_start(wos[e][:], wo[e * 128:(e + 1) * 128, :])
            bqt = [XW.tile([128, 1], F32, name=f"bq{e}") for e in range(ET)]
            bkt = [XW.tile([128, 1], F32, name=f"bk{e}") for e in range(ET)]
            for e in range(ET):
                nc.gpsimd.dma_start(bqt[e][:], bq[e])
                nc.gpsimd.dma_start(bkt[e][:], bk[e])
            # the ones columns never change: land them during the ramp
            for t in range(TT):
                dv1 = Vsb[t][:].rearrange("p (h c) -> p h c", c=66)
                nc.gpsimd.dma_start(dv1[:, :, 64:65], onesd[:])
            # Preload the exp table set off the critical path.
            warm = RT.tile([1, 2], F32, tag="warm")
            nc.vector.memset(warm[:], 0.0)
            nc.scalar.activation(warm[:], warm[:], AF.Exp)

            # -------------- projection / RoPE building blocks ------------
            def proj_qk_half(wts, bts, dst, e, c0):
                ps = PS.tile([128, 1024], F32, tag="ps")
                for d in range(DT):
                    for c in range(0, 1024, 512):
                        nc.tensor.matmul(
                            ps[:, c:c + 512],
                            wts[d][:, e * 128:(e + 1) * 128],
                            xts[d][:, c0 + c:c0 + c + 512],
                            start=(d == 0), stop=(d == DT - 1),
                            skip_group_check=True)
                nc.vector.tensor_scalar_add(
                    dst[e][:, c0:c0 + 1024], ps[:], bts[e][:])

            def proj_qk(wts, bts, dst, e):
                for c0 in range(0, L, 1024):
                    proj_qk_half(wts, bts, dst, e, c0)

            def rope(dst, e):
                # rotate_half via partition-shifted SBUF->SBUF DMA,
                # then 2 muls + add (bf16: 2x DVE).
                rs = RT.tile([128, L], BF16, tag="rs")
                tmp = RT.tile([128, L], BF16, tag="tmp")
                for g in range(4):
                    s0 = g * 32
                    d0 = s0 + 32 if g % 2 == 0 else s0 - 32
                    nc.sync.dma_start(rs[s0:s0 + 32, :],
                                      dst[e][d0:d0 + 32, :])
                nc.vector.tensor_mul(tmp[:], dst[e][:], cosbt[:])
                nc.vector.tensor_mul(rs[:], rs[:], sinbt[:])
                nc.vector.tensor_add(dst[e][:], tmp[:], rs[:])

            def v_tile(t):
                ps = PS.tile([128, EL], F32, tag="ps")
                for d in range(DT):
                    nc.tensor.matmul(
                        ps[:], xts[d][:, t * 128:(t + 1) * 128],
                        wvs[d][:],
                        start=(d == 0), stop=(d == DT - 1),
                        skip_group_check=True)
                dv = Vsb[t][:].rearrange("p (h c) -> p h c", c=66)
                nc.vector.tensor_copy(
                    dv[:, :, 0:64],
                    ps[:].rearrange("p (h c) -> p h c", c=64))

            # -------------- pre-window projections -----------------------
            # only q/k for e-tile 0 (heads 0/1) run before the exp stream
            # opens; V tiles and the whole e1 projection drip in as burst
            # filler under the exp stream.
            for wts, bts, dst in ((wqs, bqt, qT), (wks, bkt, kT)):
                proj_qk(wts, bts, dst, 0)
                rope(dst, 0)

            # One filler is consumed per k-tile step of the early bursts
            # (~1us of PE work each). V tiles stay in t-order so Vsb[kt]
            # is always ready a full burst before its first PV; the e1
            # projection (split in d-quarters to stay under the score
            # pool's prefetch slack) slots in between, RoPE chasing
            # (heads 2/3 only need e1 from burst 4 onward).
            fillers = [(lambda t=t: v_tile(t)) for t in range(TT)]

            def e1_unit(wts, bts, dst, cq):
                # self-contained 512-wide projection chunk (~2us of PE):
                # own PSUM slot, full d-accumulation, bias-add.
                ps = PS.tile([128, 512], F32, tag="ps", name="pse1")
                for d in range(DT):
                    nc.tensor.matmul(
                        ps[:], wts[d][:, 128:256],
                        xts[d][:, cq:cq + 512],
                        start=(d == 0), stop=(d == DT - 1),
                        skip_group_check=True)
                nc.vector.tensor_scalar_add(
                    dst[1][:, cq:cq + 512], ps[:], bts[1][:])

            e1w = []
            for wts, bts, dst in ((wqs, bqt, qT), (wks, bkt, kT)):
                for cq in range(0, L, 512):
                    e1w.append(lambda w=wts, b=bts, ds=dst, cq=cq:
                               e1_unit(w, b, ds, cq))
                e1w.append(lambda ds=dst: rope(ds, 1))
            for i, u in enumerate(e1w):
                fillers.insert(4 + 2 * i, u)
            # run the first few fillers before the bursts: they cover the
            # PE-idle window while the first RoPE chain (x DMA -> proj ->
            # bias -> shift-DMA -> muls) completes, so HAM never sees a
            # >3.4us idle and the first bursts start at full clock.
            for _ in range(6):
                fillers.pop(0)()

            # -------------- attention bursts -----------------------------
            bursts = [(h, q0) for h in range(NH) for q0 in range(0, L, QH)]
            ebs = {}     # (burst_idx, tk) -> eb tile
            ops = {}     # burst_idx -> op PSUM tile

            def s_exp(b, tk):
                h, q0 = bursts[b]
                e, off = divmod(h, 2)
                off *= 64
                qh = qT[e][off:off + 64, :]
                kh = kT[e][off:off + 64, :]
                sp = PS.tile([128, QH], F32, tag="ps")
                for c in range(0, QH, 512):
                    nc.tensor.matmul(
                        sp[:, c:c + 512],
                        kh[:, tk * 128:(tk + 1) * 128],
                        qh[:, q0 + c:q0 + c + 512],
                        start=True, stop=True,
                        skip_group_check=True)
                eb = EP.tile([128, QH], BF16, tag="eb")
                nc.scalar.activation(eb[:], sp[:], AF.Exp, scale=0.125)
                ebs[(b, tk)] = eb

            def pv(b, tk):
                h, q0 = bursts[b]
                if b not in ops:
                    ops[b] = PO.tile([65, QH], F32, tag="op", name=f"op{b}")
                op = ops[b]
                eb = ebs.pop((b, tk))
                for c in range(0, QH, 512):
                    nc.tensor.matmul(
                        op[:, c:c + 512],
                        Vsb[tk][:, h * 66:h * 66 + 65],
                        eb[:, c:c + 512],
                        start=(tk == 0), stop=(tk == TT - 1),
                        skip_group_check=True)

            def normalize(b, last):
                h, q0 = bursts[b]
                e, off = divmod(h, 2)
                off *= 64
                op = ops.pop(b)
                # copy raw output+denominators off PSUM immediately so
                # the accumulator frees for the next burst.
                oraw = SS.tile([65, QH], F32, tag="oraw")
                nc.vector.tensor_copy(oraw[:], op[:])
                CH = 512 if last else QH
                for c0 in range(0, QH, CH):
                    rb = SS.tile([1, CH], F32, tag="rb")
                    if last:
                        # tail-latency path: a [1, 512] reciprocal is
                        # ~1.9us on one DVE lane, cheaper than the two
                        # ~1.5us DMA round-trips of the scatter version.
                        nc.vector.reciprocal(rb[:],
                                             oraw[64:65, c0:c0 + CH])
                    else:
                        rt = SS.tile([128, CH // 128], F32, tag="rt")
                        nc.sync.dma_start(rt[:], oraw[64:65, c0:c0 + CH])
                        nc.vector.reciprocal(rt[:], rt[:])
                        nc.sync.dma_start(rb[:], rt[:])
                    rbB = SS.tile([64, CH], F32, tag="rbB")
                    nc.gpsimd.partition_broadcast(rbB[:], rb[:],
                                                  channels=64)
                    nc.vector.tensor_mul(
                        ao[e][off:off + 64, q0 + c0:q0 + c0 + CH],
                        oraw[0:64, c0:c0 + CH], rbB[:])

            def out_proj(dc, c0):
                pdt = PS.tile([128, 1024], F32, tag="ps", name="pd")
                for e in range(ET):
                    for c in range(0, 1024, 512):
                        nc.tensor.matmul(
                            pdt[:, c:c + 512],
                            wos[e][:, dc * 128:(dc + 1) * 128],
                            ao[e][:, c0 + c:c0 + c + 512],
                            start=(e == 0), stop=(e == ET - 1),
                            skip_group_check=True)
                osb = OD.tile([128, 1024], BF16, tag="osb")
                # alternate PSUM->SBUF copies between ACT (done with
                # exp) and DVE so neither serializes the drain.
                if (dc * 2 + c0 // 1024) % 2 == 0:
                    nc.scalar.activation(osb[:], pdt[:], AF.Identity)
                else:
                    nc.vector.tensor_copy(osb[:], pdt[:])
                # out DMA in two 512-col chunks: a 256KB transfer runs
                # ~11us on one DMA queue and the last tile's transfer is
                # on the kernel's critical path.
                for cc in (0, 512):
                    nc.sync.dma_start(
                        outT[dc * 128:(dc + 1) * 128,
                             c0 + cc:c0 + cc + 512],
                        osb[:, cc:cc + 512])

            for b in range(len(bursts)):
                for tk in range(TT):
                    if fillers:
                        fillers.pop(0)()
                    s_exp(b, tk)
                    if b > 0:
                        pv(b - 1, tk)
                if b > 0:
                    normalize(b - 1, last=False)
            # last burst: the PVs go first (their exps are already done,
            # whereas an out-projection tile would WAR-wait on a trailing
            # exp's score slot and, the PE being in-order, block the PVs
            # behind it); the first out-projection half (complete in ao
            # once the even bursts are normalized) then overlaps the
            # normalize of the final half.
            last = len(bursts) - 1
            for tk in range(TT):
                pv(last, tk)
            normalize(last, last=True)
            for dc in range(DT):
                out_proj(dc, 0)
            for dc in range(DT):
                out_proj(dc, 1024)

    nc.compile()
    return nc


def _rope_tables():
    inv = 1.0 / (10000.0 ** (np.arange(0, HD, 2, dtype=np.float32) / HD))
    t = np.arange(L, dtype=np.float32)
    fr = t[:, None] * inv[None, :]                    # [L, 32]
    emb = np.concatenate([fr, fr], axis=1)            # [L, 64]
    cos, sin = np.cos(emb), np.sin(emb)               # [L, 64]
    # device layout [128, L]: row p covers head-dim i = p % 64, two heads
    # stacked per 128-partition tile; sin carries the rotate_half sign.
    i = np.arange(128) % HD
    cosb = cos.T[i, :]                                # [128, L]
    sg = np.where(i < HD // 2, -1.0, 1.0).astype(np.float32)
    sinb = sin.T[i, :] * sg[:, None]
    return np.ascontiguousarray(cosb, np.float32), \
        np.ascontiguousarray(sinb, np.float32)


def _in_maps(x, q_w, q_b, k_w, k_b, v_w, o_w):
    from ml_dtypes import bfloat16
    cosb, sinb = _rope_tables()
    cosb, sinb = cosb.astype(bfloat16), sinb.astype(bfloat16)
    qwT = np.asarray(q_w, np.float32).T.astype(bfloat16)  # [D, D] eff
    kwT = np.asarray(k_w, np.float32).T.astype(bfloat16)
    vwT = np.asarray(v_w, np.float32).T.astype(bfloat16)
    owT = np.asarray(o_w, np.float32).T.astype(bfloat16)
    xTb = [np.ascontiguousarray(x[b].T).astype(bfloat16) for b in range(B)]
    maps = []
    for c in range(NCORES):
        b, hg = divmod(c, HG)
        er = slice(hg * EL, (hg + 1) * EL)
        maps.append({
            "xT": xTb[b],
            "wq": np.ascontiguousarray(qwT[:, er]),
            "wk": np.ascontiguousarray(kwT[:, er]),
            "wv": np.ascontiguousarray(vwT[:, er]),
            "wo": np.ascontiguousarray(owT[er, :]),
            "bq": np.ascontiguousarray(
                np.asarray(q_b, np.float32)[er].reshape(ET, 128, 1)),
            "bk": np.ascontiguousarray(
                np.asarray(k_b, np.float32)[er].reshape(ET, 128, 1)),
            "cosb": cosb,
            "sinb": sinb,
            "onesd": np.ones((128, NH, 1), bfloat16),
        })
    return maps


def kernel(x, q_w, q_b, k_w, k_b, v_w, v_b, o_w, o_b):
    from concourse.bass_utils import run_bass_kernel_spmd

    x = np.asarray(x, np.float32)
    assert x.shape == (B, L, D), x.shape

    if "nc" not in _cache:
        _cache["nc"] = _build()
    nc = _cache["nc"]

    in_maps = _in_maps(x, q_w, q_b, k_w, k_b, v_w, o_w)
    res = run_bass_kernel_spmd(nc, in_maps, list(range(NCORES)))

    out = np.zeros((B, L, D), np.float32)
    for c in range(NCORES):
        b = c // HG
        out[b] += res.results[c]["outT"].astype(np.float32).T
    # o_b, plus v_b's contribution (v_b flows through softmax-weighted
    # averaging unchanged, then through the out-projection).
    extra = np.asarray(o_b, np.float32) + \
        np.asarray(v_b, np.float32) @ np.asarray(o_w, np.float32).T
    out += extra[None, None, :]
    return out
